# revision 7
# baseline (speedup 1.0000x reference)
"""NetVLAD forward on 8 Trainium2 NeuronCores.

Full inputs: x [16, 128, 64, 64] f32, conv_w [64, 128], conv_b [64],
centroids [64, 128]. Output [16, 8192] f32.

Sharding: data-parallel over batch — 2 samples per core; weights replicated.

Per-sample math (C=128 channels, N=4096 positions, K=64 clusters):
  r[n]   = 1/||x[:, n]||                    (channel L2 norm)
  logits = conv_w @ x * r[n] + b            (1x1 conv on normalized x)
  a      = softmax_k(logits)
  vlad   = sum_n a[k,n] * (x[:,n]*r[n]) - centroids[k] * sum_n a[k,n]
  out    = rownorm(vlad) / sqrt(K)          (intra-norm + global norm fused;
                                             global norm == sqrt(K) exactly
                                             since rows are unit after intra)

Layout strategy per core:
  - x loaded naturally [C=128 part, N free] as float32r (PE fast-fp32 mode).
  - mm1: logits in natural layout [K, N] (lhsT = conv_w^T, FD=512).
  - PE transposes move x chunks and logit chunks into n-partitioned layout
    where softmax is a free-dim reduction and the VLAD GEMM contracts over n.
  - mm2: both samples packed block-diagonally: lhsT = [a0 | a1] [128n, 128],
    rhs = [xn0 | xn1 | ones] [128n, 257] -> psum [128, 257] holds vlad of
    both samples plus the per-cluster attention sums A_k (ones column).
"""

import numpy as np

import concourse.bass as bass
import concourse.bacc as bacc
import concourse.tile as tile
from concourse import mybir
from concourse.bass_utils import run_bass_kernel_spmd
from concourse.masks import make_identity

f32 = mybir.dt.float32
f32r = mybir.dt.float32r
f16 = mybir.dt.float16
AF = mybir.ActivationFunctionType
ALU = mybir.AluOpType
AX = mybir.AxisListType

B, C, N, K = 16, 128, 4096, 64
NCORES = 8
BS = B // NCORES          # samples per core = 2
SLAB = 1024               # n per DMA slab
NSLAB = N // SLAB         # 4
GRP = 512                 # n per mm1 group
CH = 128                  # n per chunk
LOG8 = float(np.log(0.125))


def _build():
    nc = bacc.Bacc("TRN2", target_bir_lowering=False, debug=False,
                   num_devices=NCORES)
    x_h = nc.dram_tensor("x", [BS, C, N], f32, kind="ExternalInput")
    w_h = nc.dram_tensor("conv_w", [K, C], f32, kind="ExternalInput")
    b_h = nc.dram_tensor("conv_b", [K], f32, kind="ExternalInput")
    c_h = nc.dram_tensor("centroids", [K, C], f32, kind="ExternalInput")
    o_h = nc.dram_tensor("out", [BS, K * C], f32, kind="ExternalOutput")

    with tile.TileContext(nc) as tc:
        _emit(nc, tc, x_h, w_h, b_h, c_h, o_h)
    nc.compile()
    return nc


def _emit(nc, tc, x_h, w_h, b_h, c_h, o_h):
    import contextlib
    ctx = contextlib.ExitStack()
    with ctx:
        const = ctx.enter_context(tc.tile_pool(name="const", bufs=1))
        xp = ctx.enter_context(tc.tile_pool(name="xp", bufs=4))
        sqp = ctx.enter_context(tc.tile_pool(name="sqp", bufs=3))
        l0p = ctx.enter_context(tc.tile_pool(name="l0p", bufs=3))
        e0p = ctx.enter_context(tc.tile_pool(name="e0p", bufs=12))
        esp = ctx.enter_context(tc.tile_pool(name="esp", bufs=4))
        ap_ = ctx.enter_context(tc.tile_pool(name="ap", bufs=3))
        vec = ctx.enter_context(tc.tile_pool(name="vec", bufs=8))
        fin = ctx.enter_context(tc.tile_pool(name="fin", bufs=4))
        ps_l0 = ctx.enter_context(tc.tile_pool(name="ps_l0", bufs=2, space="PSUM"))
        ps_t = ctx.enter_context(tc.tile_pool(name="ps_t", bufs=4, space="PSUM"))
        ps_v = ctx.enter_context(tc.tile_pool(name="ps_v", bufs=1, space="PSUM"))

        # ---- constants ----
        ident = const.tile([128, 128], f32, tag="ident")
        make_identity(nc, ident[:])
        id_r = const.tile([128, 128], f32r, tag="id_r")
        nc.vector.tensor_copy(out=id_r[:], in_=ident[:])

        w_sb = const.tile([K, C], f32, tag="w_sb")
        nc.sync.dma_start(out=w_sb[:], in_=w_h[:, :])
        ps_wt = ps_t.tile([128, K], f32, tag="pt")
        nc.tensor.transpose(ps_wt[:], w_sb[:], ident[0:K, 0:K])
        w_t = const.tile([C, K], f32r, tag="w_t")
        nc.vector.tensor_copy(out=w_t[:], in_=ps_wt[:])

        b_ap = b_h[:]
        b_bcast = bass.AP(tensor=b_ap.tensor, offset=b_ap.offset,
                          ap=[[0, 128], [1, K]])
        b_rep = const.tile([128, K], f32, tag="b_rep")
        nc.gpsimd.dma_start(out=b_rep[:], in_=b_bcast)
        expb = const.tile([128, K], f16, tag="expb")
        nc.scalar.activation(out=expb[:], in_=b_rep[:], func=AF.Exp)

        cent = const.tile([K, C], f32, tag="cent")
        nc.sync.dma_start(out=cent[:], in_=c_h[:, :])

        ones_f32 = const.tile([128, 1], f32, tag="ones")
        nc.vector.memset(ones_f32[:], 1.0)
        log8 = const.tile([K, 1], f32, tag="log8")
        nc.vector.memset(log8[:], LOG8)

        # persistent rhs pair tiles [xn0 | xn1 | ones], rotated manually
        NROT = 3
        xts = []
        for t in range(NROT):
            xt = const.tile([128, 264], f32r, tag=f"xtp{t}")
            nc.vector.tensor_copy(out=xt[:, 256:257], in_=ones_f32[:])
            nc.vector.tensor_copy(out=xt[:, 257:258], in_=ones_f32[:])
            xts.append(xt)

        ps_vlad = ps_v.tile([128, 264], f32, tag="vlad")

        # ---- main loop over 1024-wide slabs ----
        for g in range(NSLAB):
            x_sb = []
            for s in range(BS):
                xt_ = xp.tile([128, SLAB], f32r, tag="x_sb")
                nc.gpsimd.dma_start(
                    out=xt_[:], in_=x_h[s, :, g * SLAB:(g + 1) * SLAB])
                x_sb.append(xt_)

            # channel norms^2 for the whole slab: [128 n-part, 2, 8]
            ns_sb = vec.tile([128, BS, SLAB // CH], f32, tag="ns")
            for s in range(BS):
                sq16 = sqp.tile([128, SLAB], f16, tag="sq")
                nc.gpsimd.tensor_mul(out=sq16[:], in0=x_sb[s][:].bitcast(f32),
                                     in1=x_sb[s][:].bitcast(f32))
                nc.vector.tensor_reduce(
                    out=ns_sb[:, s, :],
                    in_=sq16[:].rearrange("p (g c) -> p g c", c=CH),
                    axis=AX.X, op=ALU.add)
            # r = 1/sqrt(ns) = exp(-0.5*ln(ns)) on ACT (single func table)
            ln_ns = vec.tile([128, BS, SLAB // CH], f32, tag="lnns")
            nc.scalar.activation(out=ln_ns[:], in_=ns_sb[:], func=AF.Ln)
            r_sl = vec.tile([128, BS, SLAB // CH], f32, tag="r")
            nc.scalar.activation(out=r_sl[:], in_=ln_ns[:], func=AF.Exp,
                                 scale=-0.5)

            for gg in range(SLAB // GRP):
                # mm1 per sample; copies pack both into one [128, 512] sbuf
                # tile (s0 rows 0-63, s1 rows 64-127) for paired transposes
                l0_sb = l0p.tile([128, GRP], f32r, tag="l0sb")
                for s in range(BS):
                    pl0 = ps_l0.tile([K, GRP], f32, tag="l0",
                                     name=f"pl0_{g}_{gg}_{s}")
                    nc.tensor.matmul(
                        pl0[:], w_t[:],
                        x_sb[s][:, gg * GRP:(gg + 1) * GRP],
                        start=True, stop=True)
                    if (gg + s) % 2 == 0:
                        nc.vector.tensor_copy(
                            out=l0_sb[s * K:(s + 1) * K, :], in_=pl0[:])
                    else:
                        nc.scalar.activation(
                            out=l0_sb[s * K:(s + 1) * K, :], in_=pl0[:],
                            func=AF.Copy)

                e0_tiles = [[None] * (GRP // CH) for _ in range(BS)]
                es_sb = [esp.tile([128, GRP // CH, K], f16, tag="es",
                                  name=f"es_{g}_{gg}_{s}")
                         for s in range(BS)]
                for j in range(GRP // CH):
                    ci = gg * (GRP // CH) + j      # chunk in slab (0..7)
                    gi = g * (SLAB // CH) + ci     # global chunk (0..31)
                    # transposes
                    plt = ps_t.tile([128, 128], f32r, tag="pt")
                    nc.tensor.transpose(
                        plt[:], l0_sb[:, j * CH:(j + 1) * CH], id_r[:])
                    pxt = []
                    for s in range(BS):
                        p = ps_t.tile([128, 128], f32r, tag="pt")
                        nc.tensor.transpose(
                            p[:], x_sb[s][:, ci * CH:(ci + 1) * CH], id_r[:])
                        pxt.append(p)
                    xt_tile = xts[gi % NROT]
                    for s in range(BS):
                        rj = r_sl[:, s, ci:ci + 1]
                        # e0 = exp(r * l0t)
                        e0 = e0p.tile([128, K], f16, tag="e0")
                        nc.scalar.activation(
                            out=e0[:], in_=plt[:, s * K:(s + 1) * K].bitcast(f32),
                            func=AF.Exp, scale=rj)
                        e0_tiles[s][j] = e0
                        # es = e0 * exp(b)   (rowsum of es = softmax denom)
                        nc.vector.tensor_mul(out=es_sb[s][:, j, :], in0=e0[:],
                                             in1=expb[:])
                        # xn = x_t * r  (psum -> sbuf, alternate DVE/ACT)
                        dst = xt_tile[:, s * 128:(s + 1) * 128]
                        if (j + s) % 2 == 0:
                            nc.scalar.activation(out=dst, in_=pxt[s][:].bitcast(f32),
                                                 func=AF.Copy, scale=rj)
                        else:
                            nc.vector.tensor_scalar(
                                out=dst, in0=pxt[s][:].bitcast(f32), scalar1=rj,
                                scalar2=None, op0=ALU.mult)

                # softmax denominators for the group
                ssum = vec.tile([128, BS, GRP // CH], f32, tag="ssum")
                for s in range(BS):
                    nc.vector.tensor_reduce(out=ssum[:, s, :], in_=es_sb[s][:],
                                            axis=AX.X, op=ALU.add)
                ln_s = vec.tile([128, BS, GRP // CH], f32, tag="lns")
                nc.scalar.activation(out=ln_s[:], in_=ssum[:], func=AF.Ln)
                rs = vec.tile([128, BS, GRP // CH], f32, tag="rs")
                nc.scalar.activation(out=rs[:], in_=ln_s[:], func=AF.Exp,
                                     scale=-1.0)

                for j in range(GRP // CH):
                    ci = gg * (GRP // CH) + j
                    gi = g * (SLAB // CH) + ci
                    a_pair = ap_.tile([128, 128], f32r, tag="a")
                    for s in range(BS):
                        # a = (e0 * 1/s) * exp(b)
                        nc.vector.scalar_tensor_tensor(
                            out=a_pair[:, s * K:(s + 1) * K],
                            in0=e0_tiles[s][j][:], scalar=rs[:, s, j:j + 1],
                            in1=expb[:], op0=ALU.mult, op1=ALU.mult)
                    nc.tensor.matmul(
                        ps_vlad[:, 0:258], a_pair[:], xts[gi % NROT][:, 0:258],
                        start=(gi == 0), stop=(gi == N // CH - 1))

        # ---- finalize: vlad -> centroid subtract -> rownorm -> out ----
        for s in range(BS):
            vsl = ps_vlad[s * K:(s + 1) * K, s * 128:s * 128 + 128]
            a_col = ps_vlad[s * K:(s + 1) * K, 256:257]
            t1 = fin.tile([K, C], f32, tag="t1")
            nc.vector.tensor_scalar(out=t1[:], in0=cent[:], scalar1=a_col,
                                    scalar2=None, op0=ALU.mult)
            t2 = fin.tile([K, C], f32, tag="t2")
            nc.vector.tensor_sub(out=t2[:], in0=vsl, in1=t1[:])
            sq2 = fin.tile([K, C], f32, tag="sq2")
            nc.vector.tensor_mul(out=sq2[:], in0=t2[:], in1=t2[:])
            rowns = fin.tile([K, 1], f32, tag="rowns")
            nc.vector.tensor_reduce(out=rowns[:], in_=sq2[:], axis=AX.X,
                                    op=ALU.add)
            ln2 = fin.tile([K, 1], f32, tag="ln2")
            nc.scalar.activation(out=ln2[:], in_=rowns[:], func=AF.Ln)
            rn = fin.tile([K, 1], f32, tag="rn")
            # 1/(8*sqrt(rowns)) = exp(-0.5*ln(rowns) + ln(1/8))
            nc.scalar.activation(out=rn[:], in_=ln2[:], func=AF.Exp,
                                 scale=-0.5, bias=log8[:])
            o_sb = fin.tile([K, C], f32, tag="osb")
            nc.vector.tensor_scalar(out=o_sb[:], in0=t2[:], scalar1=rn[:],
                                    scalar2=None, op0=ALU.mult)
            nc.sync.dma_start(
                out=o_h[s, :].rearrange("(k c) -> k c", c=C), in_=o_sb[:])


_NC = None


def kernel(x, conv_w, conv_b, centroids):
    global _NC
    if _NC is None:
        _NC = _build()
    x = np.ascontiguousarray(np.asarray(x, dtype=np.float32)).reshape(B, C, N)
    conv_w = np.asarray(conv_w, dtype=np.float32)
    conv_b = np.asarray(conv_b, dtype=np.float32)
    centroids = np.asarray(centroids, dtype=np.float32)
    in_maps = [{
        "x": x[i * BS:(i + 1) * BS],
        "conv_w": conv_w,
        "conv_b": conv_b,
        "centroids": centroids,
    } for i in range(NCORES)]
    res = run_bass_kernel_spmd(_NC, in_maps, core_ids=list(range(NCORES)))
    return np.concatenate([res.results[i]["out"] for i in range(NCORES)],
                          axis=0)
